# revision 3
# baseline (speedup 1.0000x reference)
"""Single-head causal attention (B=8, T=2048, C=1024, H=64) on 8 TRN2 NeuronCores.

Data-parallel over batch: core b computes attention for batch element b.

Device algorithm (per core); all matmul operands float16 (1 col/cycle PE rate,
half the DMA/SBUF bytes of fp32), accumulation fp32 in PSUM:
  - Inputs pre-marshalled on host (fp16): aT pre-tiled as [NCH, P, NCT, CHUNK]
    so each partition's slice of a T-quarter is one contiguous 8 KiB run (big
    DMA descriptors); Wqv = [Wq*scale | Wv] [1024, 128]; Wk [1024, 64].
  - Ramp: the framework preamble blocks all engines until ~7.2us and each
    dma_start costs ~650ns of issue time on its engine queue, so the first
    bytes land ~8.5us and early transfers run at only ~60-100 B/ns.  wqv is
    loaded in 3 pieces (c-tiles 2|3|3) and quarter 0 in 6 pieces (c-tiles
    2|1|1|1|1|2), split across the sync AND gpsimd queues so issues and
    transfers overlap; the first projection chain (c0-1) starts ~9.6us.
    4 dense warm matmuls on zeros bridge the preamble exit to the first
    chain and pull the HAM K8 clock flip earlier.
  - HAM: the PE clock is gated K=4/8 (~0.84GHz) until ~3.4us of dense
    activity flips it to K=8 (~1.4GHz); a power limiter re-throttles to K=4
    after sustained K8.  The limit point scales with how much PE work was
    done (measured 24us -> 37.5us of K8 when total PE busy dropped 16%), so
    cutting columns compounds.
  - Projections per chunk: qT/vT from lhsT=Wqv tiles (q rows 0-63, vT rows
    64-127), kT from lhsT=Wk tiles, rhs = aT C-tiles.  Chunk 0's qv chain is
    split into three PSUM groups (2|3|3 c-tiles; a group's first matmul waits
    on every input of the whole group) merged on DVE via one ScalarE staging
    copy.  Chains are kept contiguous on the tensor queue: interleaving two
    open accumulation groups breaks weight-load overlap (+230ns/matmul).
  - v natural [T-tile, 64|1]: all four per-chunk tiles via PE identity-
    operand transpose (128 cols each).  XBAR DMA-transposes were tried and
    regress: their ~1.2us serialized issue ops land right at the next
    chunk's PV start and stall it.  Column 64 is 1.0 (memset).
  - Scores transposed: sT[tk, tq] = lhsT kT tile [64, 128] x rhs qT chunk
    [64, 512] (contraction H=64); exp on ScalarE straight from PSUM.
    Diagonal k-tiles use their exact causal width; each diagonal tile's
    leading 128-col triangle is zeroed by one [128,128] band-mask multiply
    (DVE 2x fp16).
  - Attention per chunk: uniform per-k-tile groups (4 diagonal tiles first,
    then the full below-diagonal tiles), one 512-col score matmul + one exp
    + one PV per k-tile, with PV DEFERRED TWO groups (queue: S0 S1 S2 P0 S3
    P1 ...).  At K8 the PE outruns ScalarE (512-col matmul 375ns vs exp
    687ns), so a one-group deferral stalls PV ~1.4us at every exp run;
    depth 2 gives each exp ~1.1us of PE cover.  Score tiles are 1-PSUM-bank
    [128,512] with bufs=3 to support the deeper pipeline.
  - PV: outT/denom accumulate in one PSUM group per chunk: lhsT = [v | 1]
    [128, 65], rhs = expT tiles; row 64 is the softmax denominator.  No
    max-subtraction: causal logits peak ~7.2, exp <= ~1300, unnormalized
    |o| <= ~4300 and denom <= ~8800 all fit fp16.
  - NO on-device normalize: the kernel ships [o | denom] [65, T] fp16 and
    the host does out = (o/denom).T.  This removes the reciprocal/cast/
    broadcast chain (2048 PE columns + its tensor-queue stalls) and
    shortens the kernel tail to two copies + stores on parallel queues.

Timing notes (measured): minimizing total PE columns and keeping the stream
dense matters more than anything else.  fp8 DoubleRow measured only ~1.6x
per real contraction pair (cost model's 4x is wrong on this hw) and every
precision-viable fp8 construction needs residual planes that erase the gain
— fp16 everywhere is the optimum here.
"""

import sys

sys.path.insert(0, "/opt/trn_rl_repo")
sys.path.insert(0, "/root/.axon_site")

import numpy as np

import concourse.bass as bass
import concourse.mybir as mybir
import concourse.tile as tile
from concourse import bacc
from concourse import bass_utils

# If tracing is ever requested (e.g. BASS_TRACE=1), bass_utils imports
# antenv.axon_hooks, which this image lacks.  Register a ctypes-backed shim so
# that path degrades gracefully instead of raising ImportError.
try:
    from antenv import axon_hooks as _ah  # noqa: F401
except ImportError:
    try:
        import types as _types

        from trn_agent_boot.trn_boot import _ntff_profile_via_ctypes

        _mod = _types.ModuleType("antenv.axon_hooks")
        _hook = [None]
        _mod.set_axon_ntff_profile_hook = lambda h: _hook.__setitem__(0, h)
        _mod.get_axon_ntff_profile_hook = lambda: _hook[0]
        sys.modules["antenv.axon_hooks"] = _mod
        import antenv as _antenv

        _antenv.axon_hooks = _mod
        _mod.set_axon_ntff_profile_hook(
            _ntff_profile_via_ctypes("/opt/axon/libaxon_pjrt.so")
        )
    except Exception:
        pass

B, T, C, H = 8, 2048, 1024, 64
P = 128
NCT = C // P          # 8 C-tiles (contraction)
CHUNK = 512           # q-columns per chunk
NCH = T // CHUNK      # 4 chunks
NKT = T // P          # 16 k-tiles
SCALE = H ** -0.5
FP = mybir.dt.float32
F16 = mybir.dt.float16

QV0_CHAINS = [2, 3, 3]                  # chunk-0 qv PSUM chains (c-tile counts)

_cache = {}


def build_program():
    nc = bacc.Bacc("TRN2", target_bir_lowering=False, debug=False)

    aT = nc.dram_tensor("aT", [NCH, P, NCT, CHUNK], F16, kind="ExternalInput").ap()
    wqv = nc.dram_tensor("wqv", [C, 2 * H], F16, kind="ExternalInput").ap()
    wk = nc.dram_tensor("wk", [C, H], F16, kind="ExternalInput").ap()
    idh = nc.dram_tensor("idh", [P, H], F16, kind="ExternalInput").ap()
    m4 = nc.dram_tensor("m4", [P, P], F16, kind="ExternalInput").ap()
    outT = nc.dram_tensor("outT", [H + 1, T], F16, kind="ExternalOutput").ap()

    wqv_r = wqv.rearrange("(ko p) m -> p ko m", p=P)

    with tile.TileContext(nc) as tc:
        with (
            tc.tile_pool(name="const", bufs=1) as const_pool,
            tc.tile_pool(name="at", bufs=1) as at_pool,
            tc.tile_pool(name="qv", bufs=1) as qv_pool,
            tc.tile_pool(name="kt", bufs=1) as kt_pool,
            tc.tile_pool(name="v1", bufs=NKT) as v1_pool,
            tc.tile_pool(name="es", bufs=4) as e_pool,
            tc.tile_pool(name="stage", bufs=4) as stage_pool,
            tc.tile_pool(name="out", bufs=1) as out_pool,
            tc.tile_pool(name="ps_s", bufs=3, space="PSUM") as s_psum,
            tc.tile_pool(name="ps_proj", bufs=2, space="PSUM") as proj_psum,
            tc.tile_pool(name="ps_pv", bufs=1, space="PSUM") as pv_psum,
            tc.tile_pool(name="ps_small", bufs=1, space="PSUM") as small_psum,
        ):
            # ---- warm the ACT exp table + the PE clock during the DMA window
            warm = const_pool.tile([P, 8], FP, tag="warm")
            nc.scalar.activation(
                warm[:], warm[:], mybir.ActivationFunctionType.Exp
            )
            warm2 = const_pool.tile([P, CHUNK], F16, tag="warm2")
            nc.vector.memset(warm2[:], 0.0)
            warm_ps = small_psum.tile([P, CHUNK], FP, tag="small")
            for _ in range(4):
                nc.tensor.matmul(
                    warm_ps[:], warm2[:, :P], warm2[:], start=True, stop=True,
                )

            # ---- input DMA.  Tile dependencies are whole-tile: a consumer
            # waits for EVERY dma_start into its tile, and a PSUM group's
            # first matmul waits on every input of the whole group.  Early
            # data is loaded as several SEPARATE piece-tiles, split across
            # the sync and gpsimd queues so the ~650ns issue ops and the
            # transfers overlap.  Bulk quarters ride the sync queue only: a
            # second hwdge queue on the bulk floods the 16 shared DMA rings
            # and starves the ramp-critical early pieces (measured). ----
            wqv_sb = {}
            at_sb = {}             # (j, piece) -> tile
            q0_pieces = []         # (start c-tile, tile)

            def load_wqv_piece(pi, c0, cn, eng):
                t_ = const_pool.tile([P, cn, 2 * H], F16, tag=f"wqv{pi}")
                eng.dma_start(t_[:], wqv_r[:, c0 : c0 + cn, :])
                wqv_sb[pi] = (c0, t_)

            def wqv_tile(c):
                for c0, t_ in wqv_sb.values():
                    if c0 <= c < c0 + t_.shape[1]:
                        return t_[:, c - c0, :]
                raise KeyError(c)

            def load_q0_piece(pi, c0, cn, eng):
                t_ = at_pool.tile([P, cn, CHUNK], F16, tag=f"at0_{pi}")
                eng.dma_start(t_[:], aT[0, :, c0 : c0 + cn, :])
                q0_pieces.append((c0, t_))

            def at_tile(j, c):
                if j == 0:
                    for c0, t_ in q0_pieces:
                        if c0 <= c < c0 + t_.shape[1]:
                            return t_[:, c - c0, :]
                    raise KeyError(c)
                step = at_step[j]
                return at_sb[(j, c // step)][:, c % step, :]

            # need order; chain A (c0-1) gates on sync #1-2, B (c2-4) on
            # wqvp1/c3 (sync) + c2/c4 (gpsimd), C (c5-7) on c5 (sync) +
            # wqvp2/c67 (gpsimd)
            load_wqv_piece(0, 0, 2, nc.sync)
            load_q0_piece(0, 0, 2, nc.sync)
            load_q0_piece(1, 2, 1, nc.gpsimd)
            load_wqv_piece(1, 2, 3, nc.sync)
            load_q0_piece(2, 3, 1, nc.sync)
            load_q0_piece(3, 4, 1, nc.gpsimd)
            load_wqv_piece(2, 5, 3, nc.gpsimd)
            load_q0_piece(4, 5, 1, nc.sync)
            load_q0_piece(5, 6, 2, nc.gpsimd)
            wk_sb = const_pool.tile([P, NCT, H], F16, tag="wk")
            nc.sync.dma_start(wk_sb[:], wk.rearrange("(ko p) m -> p ko m", p=P))

            at_step = {}

            def load_quarter(j, pieces=1):
                step = NCT // pieces
                at_step[j] = step
                for h in range(pieces):
                    t_ = at_pool.tile([P, step, CHUNK], F16, tag=f"at{j}_{h}")
                    nc.sync.dma_start(
                        t_[:], aT[j, :, h * step : (h + 1) * step, :]
                    )
                    at_sb[(j, h)] = t_

            load_quarter(1, pieces=2)
            for j in range(2, NCH):
                load_quarter(j)

            # idle-time consts on the gpsimd queue (after the ramp pieces)
            idh_sb = const_pool.tile([P, H], F16, tag="idh")
            nc.gpsimd.dma_start(idh_sb[:], idh[:])
            m4_sb = const_pool.tile([P, P], F16, tag="m4")
            nc.gpsimd.dma_start(m4_sb[:], m4[:])

            qv_sb = qv_pool.tile([P, T], F16, tag="qv")   # q rows 0-63, vT rows 64-127
            kT_sb = kt_pool.tile([H, T], F16, tag="kt")
            o65_sb = out_pool.tile([H + 1, T], F16, tag="ot")
            v1 = {}

            def proj(j):
                # NOTE: keep each PSUM accumulation chain contiguous on the
                # tensor queue — interleaving two open accumulation groups
                # costs ~230ns/matmul (weight-load overlap breaks)
                cs = slice(j * CHUNK, (j + 1) * CHUNK)
                if j == 0:
                    # Ramp chunk: three qv chains sized to the piece arrival
                    # order
                    ps = []
                    c0 = 0
                    for ci, cn in enumerate(QV0_CHAINS):
                        pool = small_psum if ci == 2 else proj_psum
                        p_ = pool.tile(
                            [P, CHUNK], FP,
                            tag="small" if ci == 2 else "proj",
                        )
                        for c in range(c0, c0 + cn):
                            nc.tensor.matmul(
                                p_[:], wqv_tile(c), at_tile(j, c),
                                start=(c == c0), stop=(c == c0 + cn - 1),
                            )
                        ps.append(p_)
                        c0 += cn
                    # merge: DVE may read only ONE PSUM operand per op; stage
                    # ps[1] via the idle ScalarE first
                    qb_sb = stage_pool.tile([P, CHUNK], FP, tag="qb")
                    nc.scalar.copy(qb_sb[:], ps[1][:])
                    t0_sb = stage_pool.tile([P, CHUNK], FP, tag="t0")
                    nc.vector.tensor_add(t0_sb[:], ps[0][:], qb_sb[:])
                    nc.vector.tensor_add(qv_sb[:, cs], ps[2][:], t0_sb[:])
                    ps_k = proj_psum.tile([P, CHUNK], FP, tag="proj")
                    for c in range(NCT):
                        nc.tensor.matmul(
                            ps_k[:H], wk_sb[:, c, :], at_tile(j, c),
                            start=(c == 0), stop=(c == NCT - 1),
                        )
                else:
                    ps_qv = proj_psum.tile([P, CHUNK], FP, tag="proj")
                    for c in range(NCT):
                        nc.tensor.matmul(
                            ps_qv[:], wqv_tile(c), at_tile(j, c),
                            start=(c == 0), stop=(c == NCT - 1),
                        )
                    ps_k = proj_psum.tile([P, CHUNK], FP, tag="proj")
                    for c in range(NCT):
                        nc.tensor.matmul(
                            ps_k[:H], wk_sb[:, c, :], at_tile(j, c),
                            start=(c == 0), stop=(c == NCT - 1),
                        )
                    nc.vector.tensor_copy(qv_sb[:, cs], ps_qv[:])
                # kT copy on ScalarE: overlaps the DVE qv copy, so scores
                # for the next chunk are not gated on two serial DVE ops
                nc.scalar.copy(kT_sb[:, cs], ps_k[:H])

            proj(0)
            for j in range(NCH):
                cs = slice(j * CHUNK, (j + 1) * CHUNK)

                # ---- v natural tiles ([v | 1]) via PE transpose ----
                for r in range(4):
                    kt = 4 * j + r
                    vt = v1_pool.tile([P, H + 1], F16, tag="v1")
                    nc.vector.memset(vt[:, H : H + 1], 1.0)
                    ps_t = small_psum.tile([P, H], F16, tag="small")
                    nc.tensor.transpose(
                        ps_t[:],
                        qv_sb[H:P, kt * P : (kt + 1) * P],
                        idh_sb[H:P, :],
                    )
                    nc.vector.tensor_copy(vt[:, :H], ps_t[:])
                    v1[kt] = vt

                # ---- attention: uniform per-k-tile groups, PV deferred two
                # groups so each exp has ~3 matmuls of PE cover (queue:
                # S0 S1 S2 P0 S3 P1 ...).  4 diagonal k-tiles (exact causal
                # width) first, then full k-tiles. ----
                ps_o = pv_psum.tile([H + 1, CHUNK], FP, tag="pv")
                order = [(4 * j + r, P * r) for r in range(4)]
                order += [(kt, None) for kt in range(4 * j)]
                n_pv = len(order)
                n_emit = 0
                pend = []

                def emit_pv():
                    nonlocal n_emit
                    args = pend.pop(0)
                    nc.tensor.matmul(
                        *args, start=(n_emit == 0), stop=(n_emit == n_pv - 1)
                    )
                    n_emit += 1

                for kt, off in order:
                    diag = off is not None
                    ncols = CHUNK - off if diag else CHUNK
                    qlo = j * CHUNK + (off or 0)
                    ps_s = s_psum.tile([P, CHUNK], FP, tag="s")
                    nc.tensor.matmul(
                        ps_s[:, :ncols],
                        kT_sb[:, kt * P : (kt + 1) * P],
                        qv_sb[:H, qlo : (j + 1) * CHUNK],
                        start=True, stop=True,
                    )
                    e_sb = e_pool.tile([P, CHUNK], F16, tag="e")
                    nc.scalar.activation(
                        e_sb[:, :ncols], ps_s[:, :ncols],
                        mybir.ActivationFunctionType.Exp,
                    )
                    if diag:
                        # zero the above-causal triangle in the leading
                        # 128-col block
                        nc.vector.tensor_mul(
                            e_sb[:, :P], e_sb[:, :P], m4_sb[:],
                        )
                    if len(pend) == 2:
                        emit_pv()
                    pend.append(
                        (
                            ps_o[:, off:] if diag else ps_o[:],
                            v1[kt][:],
                            e_sb[:, :ncols],
                        )
                    )
                while pend:
                    emit_pv()

                # ---- ship [o | denom] unnormalized; the host divides.
                # Early o65 copy releases the PV bank for the next chunk;
                # emitted BEFORE proj(j+1) so the DVE does it first. ----
                if j == NCH - 1:
                    # kernel tail: split halves across engines + parallel
                    # HWDGE queues so the copies and stores drain in parallel
                    HC = CHUNK // 2
                    h0 = slice(j * CHUNK, j * CHUNK + HC)
                    h1 = slice(j * CHUNK + HC, (j + 1) * CHUNK)
                    nc.vector.tensor_copy(o65_sb[:, h0], ps_o[:, :HC])
                    nc.sync.dma_start(outT[:, h0], o65_sb[:, h0])
                    nc.scalar.copy(o65_sb[:, h1], ps_o[:, HC:])
                    nc.gpsimd.dma_start(outT[:, h1], o65_sb[:, h1])
                else:
                    nc.vector.tensor_copy(o65_sb[:, cs], ps_o[:])
                    nc.gpsimd.dma_start(outT[:, cs], o65_sb[:, cs])
                    # ---- next chunk's projections: keep the tensor queue
                    # fed while the copies run on Vector/Scalar ----
                    proj(j + 1)

    nc.compile()
    return nc


def _marshal(a, Wk, Wq, Wv):
    # [B, NCH, P, NCT, CHUNK]: quarter-major, partition-major within quarter,
    # so each partition's slice of a quarter is one contiguous 8 KiB run
    aT = np.ascontiguousarray(
        a.transpose(0, 2, 1)
        .reshape(B, NCT, P, NCH, CHUNK)
        .transpose(0, 3, 2, 1, 4)
        .astype(np.float16)
    )
    wqv = np.ascontiguousarray(
        np.concatenate([Wq * np.float32(SCALE), Wv], axis=1).astype(np.float16)
    )                                                          # [C, 128]
    idh = np.zeros((P, H), np.float16)
    idh[H:P, :] = np.eye(H, dtype=np.float16)
    p = np.arange(P)[:, None]
    g = np.arange(P)[None, :]
    m4 = (g >= p).astype(np.float16)
    return aT, wqv, np.ascontiguousarray(Wk.astype(np.float16)), idh, m4


def kernel(a, Wk, Wq, Wv):
    a = np.asarray(a, np.float32)
    Wk = np.asarray(Wk, np.float32)
    Wq = np.asarray(Wq, np.float32)
    Wv = np.asarray(Wv, np.float32)
    if "nc" not in _cache:
        _cache["nc"] = build_program()
    nc = _cache["nc"]

    aT, wqv, wk, idh, m4 = _marshal(a, Wk, Wq, Wv)
    in_maps = [
        {"aT": aT[b], "wqv": wqv, "wk": wk, "idh": idh, "m4": m4}
        for b in range(B)
    ]
    res = bass_utils.run_bass_kernel_spmd(nc, in_maps, core_ids=list(range(B)))
    outs = []
    for b in range(B):
        o = np.asarray(res.results[b]["outT"], np.float32)   # [65, T]
        outs.append((o[:H] / o[H : H + 1]).T)
    return np.stack(outs).astype(np.float32)


# revision 4
# speedup vs baseline: 1.1058x; 1.1058x over previous
"""Single-head causal attention (B=8, T=2048, C=1024, H=64) on 8 TRN2 NeuronCores.

Data-parallel over batch: core b computes attention for batch element b.

Device algorithm (per core); all matmul operands float16 (1 col/cycle PE rate,
half the DMA/SBUF bytes of fp32), accumulation fp32 in PSUM:
  - Inputs pre-marshalled on host (fp16): aT pre-tiled as [NCH, P, NCT, CHUNK]
    and weights as [P, NCT, .] so every DMA descriptor is a 1-2KiB contiguous
    per-partition run; Wqv = [Wq*scale | Wv] [P, NCT, 128]; Wk [P, NCT, 64].
  - Ramp: the framework preamble blocks all engines until ~7.2us, dma_start
    costs ~650ns of issue time on its queue, and early DMA delivers only
    ~85-150 B/ns, so quarter 0 + weights (1.4 MiB) take ~7us to land.  The
    ramp is DMA-paced: quarter 0 is loaded as 8 single-c-tile pieces split
    across the sync and gpsimd queues, and chunk 0 runs its K CHAIN FIRST in
    piece-ARRIVAL order (PSUM accumulation is order-free) so the PE consumes
    pieces as they land; the qv chain follows when all pieces are present.
    5 dense warm matmuls on zeros bridge the preamble exit to the first
    piece and start the HAM activity window.
  - HAM: the PE clock is gated K=4/8 (~0.84GHz) until ~3.4us of dense
    activity flips it to K=8 (~1.4GHz); a power limiter re-throttles to K=4
    after sustained K8.  The limit point scales with total PE work done
    (measured 24us -> 37.5us of K8 when total PE busy dropped 16%), so
    cutting columns and gaps compounds.
  - Projections per chunk: qT/vT from lhsT=Wqv tiles (q rows 0-63, vT rows
    64-127), kT from lhsT=Wk tiles, rhs = aT C-tiles.  Chains are kept
    contiguous on the tensor queue: interleaving two open accumulation
    groups breaks weight-load overlap (+230ns/matmul).
  - v natural [T-tile, 64|1]: all four per-chunk tiles via PE identity-
    operand transpose (128 cols each).  XBAR DMA-transposes regress: their
    ~1.2us serialized issue ops land right at the next chunk's PV start and
    stall it.  Column 64 is 1.0 (memset).
  - Scores transposed: sT[tk, tq] = lhsT kT tile [64, 128] x rhs qT chunk
    [64, 512] (contraction H=64); exp on ScalarE straight from PSUM.
    Diagonal k-tiles use their exact causal width; each diagonal tile's
    leading 128-col triangle is zeroed by one [128,128] band-mask multiply
    (DVE 2x fp16).
  - Attention per chunk: uniform per-k-tile groups — full below-diagonal
    tiles first, then the 4 diagonal tiles LAST (their exps are short, so
    the final deferred PVs are barely exposed at the kernel tail).  One
    512-col score matmul + one exp + one PV per k-tile, with PV DEFERRED
    TWO groups (queue: S0 S1 S2 P0 S3 P1 ...).  At K8 the PE outruns
    ScalarE (512-col matmul 375ns vs exp 687ns); depth 2 gives each exp
    ~1.1us of PE cover.  Score tiles are 1-PSUM-bank [128,512], bufs=3.
  - PV: outT/denom accumulate in one PSUM group per chunk: lhsT = [v | 1]
    [128, 65], rhs = expT tiles; row 64 is the softmax denominator.  No
    max-subtraction: causal logits peak ~7.2, exp <= ~1300, unnormalized
    |o| <= ~4300 and denom <= ~8800 all fit fp16.
  - NO on-device normalize: the kernel ships [o | denom] [65, T] fp16 and
    the host does out = (o/denom).T.  This removes the reciprocal/cast/
    broadcast chain (2048 PE columns + its tensor-queue stalls); the tail
    is two parallel copies (DVE+ScalarE) and stores (sync+gpsimd queues).

Timing notes (measured): minimizing total PE columns and keeping the stream
dense matters more than anything else.  fp8 DoubleRow measured only ~1.6x
per real contraction pair (cost model's 4x is wrong on this hw) and every
precision-viable fp8 construction needs residual planes that erase the gain
— fp16 everywhere is the optimum here.
"""

import sys

sys.path.insert(0, "/opt/trn_rl_repo")
sys.path.insert(0, "/root/.axon_site")

import numpy as np

import concourse.bass as bass
import concourse.mybir as mybir
import concourse.tile as tile
from concourse import bacc
from concourse import bass_utils

# If tracing is ever requested (e.g. BASS_TRACE=1), bass_utils imports
# antenv.axon_hooks, which this image lacks.  Register a ctypes-backed shim so
# that path degrades gracefully instead of raising ImportError.
try:
    from antenv import axon_hooks as _ah  # noqa: F401
except ImportError:
    try:
        import types as _types

        from trn_agent_boot.trn_boot import _ntff_profile_via_ctypes

        _mod = _types.ModuleType("antenv.axon_hooks")
        _hook = [None]
        _mod.set_axon_ntff_profile_hook = lambda h: _hook.__setitem__(0, h)
        _mod.get_axon_ntff_profile_hook = lambda: _hook[0]
        sys.modules["antenv.axon_hooks"] = _mod
        import antenv as _antenv

        _antenv.axon_hooks = _mod
        _mod.set_axon_ntff_profile_hook(
            _ntff_profile_via_ctypes("/opt/axon/libaxon_pjrt.so")
        )
    except Exception:
        pass

B, T, C, H = 8, 2048, 1024, 64
P = 128
NCT = C // P          # 8 C-tiles (contraction)
CHUNK = 512           # q-columns per chunk
NCH = T // CHUNK      # 4 chunks
NKT = T // P          # 16 k-tiles
SCALE = H ** -0.5
FP = mybir.dt.float32
F16 = mybir.dt.float16

# chunk-0 piece queues and the k-chain's piece-arrival order
Q0_SYNC = [0, 1, 4, 5]
Q0_GP = [2, 3, 6, 7]
K0_ORDER = [0, 2, 1, 3, 4, 6, 5, 7]

_cache = {}


def build_program():
    nc = bacc.Bacc("TRN2", target_bir_lowering=False, debug=False)

    aT = nc.dram_tensor("aT", [NCH, P, NCT, CHUNK], F16, kind="ExternalInput").ap()
    wqv = nc.dram_tensor("wqv", [P, NCT, 2 * H], F16, kind="ExternalInput").ap()
    wk = nc.dram_tensor("wk", [P, NCT, H], F16, kind="ExternalInput").ap()
    idh = nc.dram_tensor("idh", [P, H], F16, kind="ExternalInput").ap()
    m4 = nc.dram_tensor("m4", [P, P], F16, kind="ExternalInput").ap()
    outT = nc.dram_tensor("outT", [H + 1, T], F16, kind="ExternalOutput").ap()

    with tile.TileContext(nc) as tc:
        with (
            tc.tile_pool(name="const", bufs=1) as const_pool,
            tc.tile_pool(name="at", bufs=1) as at_pool,
            tc.tile_pool(name="qv", bufs=1) as qv_pool,
            tc.tile_pool(name="kt", bufs=1) as kt_pool,
            tc.tile_pool(name="v1", bufs=NKT) as v1_pool,
            tc.tile_pool(name="es", bufs=4) as e_pool,
            tc.tile_pool(name="out", bufs=1) as out_pool,
            tc.tile_pool(name="ps_s", bufs=3, space="PSUM") as s_psum,
            tc.tile_pool(name="ps_proj", bufs=2, space="PSUM") as proj_psum,
            tc.tile_pool(name="ps_pv", bufs=1, space="PSUM") as pv_psum,
            tc.tile_pool(name="ps_small", bufs=1, space="PSUM") as small_psum,
        ):
            # ---- warm the ACT exp table + the PE clock during the DMA window
            warm = const_pool.tile([P, 8], FP, tag="warm")
            nc.scalar.activation(
                warm[:], warm[:], mybir.ActivationFunctionType.Exp
            )
            warm2 = const_pool.tile([P, CHUNK], F16, tag="warm2")
            nc.vector.memset(warm2[:], 0.0)
            warm_ps = small_psum.tile([P, CHUNK], FP, tag="small")
            for _ in range(5):
                nc.tensor.matmul(
                    warm_ps[:], warm2[:, :P], warm2[:], start=True, stop=True,
                )

            # ---- input DMA.  Early bytes are precious: wk first on sync
            # (gates the arrival-ordered k chain), wqv on gpsimd, quarter-0
            # single-c-tile pieces alternating between the queues.  Bulk
            # quarters ride the sync queue only: a second hwdge queue on the
            # bulk floods the 16 shared DMA rings (measured). ----
            at_sb = {}             # (j, piece-or-ctile) -> tile

            wk_sb = const_pool.tile([P, NCT, H], F16, tag="wk")
            nc.sync.dma_start(wk_sb[:], wk[:])
            wqv_sb = const_pool.tile([P, NCT, 2 * H], F16, tag="wqv")
            nc.gpsimd.dma_start(wqv_sb[:], wqv[:])
            for c_s, c_g in zip(Q0_SYNC, Q0_GP):
                for c, eng in ((c_s, nc.sync), (c_g, nc.gpsimd)):
                    t_ = at_pool.tile([P, 1, CHUNK], F16, tag=f"at0_{c}")
                    eng.dma_start(t_[:], aT[0, :, c : c + 1, :])
                    at_sb[(0, c)] = t_

            at_step = {0: 1}

            def at_tile(j, c):
                step = at_step[j]
                return at_sb[(j, c // step)][:, c % step, :]

            def load_quarter(j, pieces=1):
                step = NCT // pieces
                at_step[j] = step
                for h in range(pieces):
                    t_ = at_pool.tile([P, step, CHUNK], F16, tag=f"at{j}_{h}")
                    nc.sync.dma_start(
                        t_[:], aT[j, :, h * step : (h + 1) * step, :]
                    )
                    at_sb[(j, h)] = t_

            load_quarter(1, pieces=2)
            for j in range(2, NCH):
                load_quarter(j)

            # idle-time consts on the gpsimd queue (after the ramp pieces)
            idh_sb = const_pool.tile([P, H], F16, tag="idh")
            nc.gpsimd.dma_start(idh_sb[:], idh[:])
            m4_sb = const_pool.tile([P, P], F16, tag="m4")
            nc.gpsimd.dma_start(m4_sb[:], m4[:])

            qv_sb = qv_pool.tile([P, T], F16, tag="qv")   # q rows 0-63, vT rows 64-127
            kT_sb = kt_pool.tile([H, T], F16, tag="kt")
            o65_sb = out_pool.tile([H + 1, T], F16, tag="ot")
            v1 = {}

            def proj(j):
                # NOTE: keep each PSUM accumulation chain contiguous on the
                # tensor queue — interleaving two open accumulation groups
                # costs ~230ns/matmul (weight-load overlap breaks)
                cs = slice(j * CHUNK, (j + 1) * CHUNK)
                if j == 0:
                    # DMA-paced ramp chunk: k chain first, consuming pieces
                    # in arrival order; kT copy right after so scores gate
                    # clears early; qv chain once every piece is present.
                    ps_k = proj_psum.tile([P, CHUNK], FP, tag="proj")
                    for i, c in enumerate(K0_ORDER):
                        nc.tensor.matmul(
                            ps_k[:H], wk_sb[:, c, :], at_tile(j, c),
                            start=(i == 0), stop=(i == NCT - 1),
                        )
                    nc.scalar.copy(kT_sb[:, cs], ps_k[:H])
                    ps_qv = proj_psum.tile([P, CHUNK], FP, tag="proj")
                    for c in range(NCT):
                        nc.tensor.matmul(
                            ps_qv[:], wqv_sb[:, c, :], at_tile(j, c),
                            start=(c == 0), stop=(c == NCT - 1),
                        )
                    nc.vector.tensor_copy(qv_sb[:, cs], ps_qv[:])
                else:
                    ps_qv = proj_psum.tile([P, CHUNK], FP, tag="proj")
                    for c in range(NCT):
                        nc.tensor.matmul(
                            ps_qv[:], wqv_sb[:, c, :], at_tile(j, c),
                            start=(c == 0), stop=(c == NCT - 1),
                        )
                    ps_k = proj_psum.tile([P, CHUNK], FP, tag="proj")
                    for c in range(NCT):
                        nc.tensor.matmul(
                            ps_k[:H], wk_sb[:, c, :], at_tile(j, c),
                            start=(c == 0), stop=(c == NCT - 1),
                        )
                    nc.vector.tensor_copy(qv_sb[:, cs], ps_qv[:])
                    # kT copy on ScalarE: overlaps the DVE qv copy, so
                    # scores for the next chunk are not gated on two serial
                    # DVE ops
                    nc.scalar.copy(kT_sb[:, cs], ps_k[:H])

            proj(0)
            for j in range(NCH):
                cs = slice(j * CHUNK, (j + 1) * CHUNK)

                # ---- v natural tiles ([v | 1]) via PE transpose ----
                for r in range(4):
                    kt = 4 * j + r
                    vt = v1_pool.tile([P, H + 1], F16, tag="v1")
                    nc.vector.memset(vt[:, H : H + 1], 1.0)
                    ps_t = small_psum.tile([P, H], F16, tag="small")
                    nc.tensor.transpose(
                        ps_t[:],
                        qv_sb[H:P, kt * P : (kt + 1) * P],
                        idh_sb[H:P, :],
                    )
                    nc.vector.tensor_copy(vt[:, :H], ps_t[:])
                    v1[kt] = vt

                # ---- attention: uniform per-k-tile groups, PV deferred two
                # groups (queue: S0 S1 S2 P0 S3 P1 ...).  Full k-tiles
                # first, the 4 diagonal tiles (short exps) last. ----
                ps_o = pv_psum.tile([H + 1, CHUNK], FP, tag="pv")
                order = [(kt, None) for kt in range(4 * j)]
                order += [(4 * j + r, P * r) for r in range(4)]
                n_pv = len(order)
                n_emit = 0
                pend = []

                def emit_pv():
                    nonlocal n_emit
                    args = pend.pop(0)
                    nc.tensor.matmul(
                        *args, start=(n_emit == 0), stop=(n_emit == n_pv - 1)
                    )
                    n_emit += 1

                for kt, off in order:
                    diag = off is not None
                    ncols = CHUNK - off if diag else CHUNK
                    qlo = j * CHUNK + (off or 0)
                    ps_s = s_psum.tile([P, CHUNK], FP, tag="s")
                    nc.tensor.matmul(
                        ps_s[:, :ncols],
                        kT_sb[:, kt * P : (kt + 1) * P],
                        qv_sb[:H, qlo : (j + 1) * CHUNK],
                        start=True, stop=True,
                    )
                    e_sb = e_pool.tile([P, CHUNK], F16, tag="e")
                    nc.scalar.activation(
                        e_sb[:, :ncols], ps_s[:, :ncols],
                        mybir.ActivationFunctionType.Exp,
                    )
                    if diag:
                        # zero the above-causal triangle in the leading
                        # 128-col block
                        nc.vector.tensor_mul(
                            e_sb[:, :P], e_sb[:, :P], m4_sb[:],
                        )
                    if len(pend) == 2:
                        emit_pv()
                    pend.append(
                        (
                            ps_o[:, off:] if diag else ps_o[:],
                            v1[kt][:],
                            e_sb[:, :ncols],
                        )
                    )
                while pend:
                    emit_pv()

                # ---- ship [o | denom] unnormalized; the host divides.
                # Early o65 copy releases the PV bank for the next chunk;
                # emitted BEFORE proj(j+1) so the DVE does it first. ----
                if j == NCH - 1:
                    # kernel tail: split halves across engines + parallel
                    # HWDGE queues so copies and stores drain in parallel
                    HC = CHUNK // 2
                    h0 = slice(j * CHUNK, j * CHUNK + HC)
                    h1 = slice(j * CHUNK + HC, (j + 1) * CHUNK)
                    nc.vector.tensor_copy(o65_sb[:, h0], ps_o[:, :HC])
                    nc.scalar.copy(o65_sb[:, h1], ps_o[:, HC:])
                    nc.sync.dma_start(outT[:, h0], o65_sb[:, h0])
                    nc.gpsimd.dma_start(outT[:, h1], o65_sb[:, h1])
                else:
                    nc.vector.tensor_copy(o65_sb[:, cs], ps_o[:])
                    nc.gpsimd.dma_start(outT[:, cs], o65_sb[:, cs])
                    # ---- next chunk's projections: keep the tensor queue
                    # fed while the copies run on Vector/Scalar ----
                    proj(j + 1)

    nc.compile()
    return nc


def _marshal(a, Wk, Wq, Wv):
    # [B, NCH, P, NCT, CHUNK]: quarter-major, partition-major within quarter,
    # so each partition's slice of a quarter is one contiguous 8 KiB run
    aT = np.ascontiguousarray(
        a.transpose(0, 2, 1)
        .reshape(B, NCT, P, NCH, CHUNK)
        .transpose(0, 3, 2, 1, 4)
        .astype(np.float16)
    )
    # weights pre-tiled [P, NCT, .] so each partition's DMA run is contiguous
    wqv = np.concatenate(
        [Wq * np.float32(SCALE), Wv], axis=1
    ).astype(np.float16).reshape(NCT, P, 2 * H).transpose(1, 0, 2)
    wkt = Wk.astype(np.float16).reshape(NCT, P, H).transpose(1, 0, 2)
    idh = np.zeros((P, H), np.float16)
    idh[H:P, :] = np.eye(H, dtype=np.float16)
    p = np.arange(P)[:, None]
    g = np.arange(P)[None, :]
    m4 = (g >= p).astype(np.float16)
    return (
        aT,
        np.ascontiguousarray(wqv),
        np.ascontiguousarray(wkt),
        idh,
        m4,
    )


def kernel(a, Wk, Wq, Wv):
    a = np.asarray(a, np.float32)
    Wk = np.asarray(Wk, np.float32)
    Wq = np.asarray(Wq, np.float32)
    Wv = np.asarray(Wv, np.float32)
    if "nc" not in _cache:
        _cache["nc"] = build_program()
    nc = _cache["nc"]

    aT, wqv, wk, idh, m4 = _marshal(a, Wk, Wq, Wv)
    in_maps = [
        {"aT": aT[b], "wqv": wqv, "wk": wk, "idh": idh, "m4": m4}
        for b in range(B)
    ]
    res = bass_utils.run_bass_kernel_spmd(nc, in_maps, core_ids=list(range(B)))
    outs = []
    for b in range(B):
        o = np.asarray(res.results[b]["outT"], np.float32)   # [65, T]
        outs.append((o[:H] / o[H : H + 1]).T)
    return np.stack(outs).astype(np.float32)


# revision 6
# speedup vs baseline: 1.1376x; 1.0287x over previous
"""Single-head causal attention (B=8, T=2048, C=1024, H=64) on 8 TRN2 NeuronCores.

Data-parallel over batch: core b computes attention for batch element b.

Device algorithm (per core); all matmul operands float16 (1 col/cycle PE rate,
half the DMA/SBUF bytes of fp32), accumulation fp32 in PSUM:
  - Inputs pre-marshalled on host (fp16): aT pre-tiled as [NCH, P, NCT, CHUNK]
    and weights as [P, NCT, .] so every DMA descriptor is a 1-2KiB contiguous
    per-partition run; Wqv = [Wq*scale | Wv] [P, NCT, 128]; Wk [P, NCT, 64].
  - Ramp: the framework preamble blocks all engines until ~7.2us, dma_start
    costs ~650ns of issue time on its queue, and early DMA delivers only
    ~85-150 B/ns, so quarter 0 + weights (1.4 MiB) take ~7us to land.  The
    ramp is DMA-paced: quarter 0 is loaded as 8 single-c-tile pieces split
    across the sync and gpsimd queues, and chunk 0 runs its K CHAIN FIRST in
    piece-ARRIVAL order (PSUM accumulation is order-free) so the PE consumes
    pieces as they land; the qv chain follows when all pieces are present.
    5 dense warm matmuls on zeros bridge the preamble exit to the first
    piece and start the HAM activity window.
  - HAM: the PE clock is gated K=4/8 (~0.84GHz) until ~3.4us of dense
    activity flips it to K=8 (~1.4GHz); a power limiter re-throttles to K=4
    after sustained K8.  The limit point scales with total PE work done
    (measured 24us -> 37.5us of K8 when total PE busy dropped 16%), so
    cutting columns and gaps compounds.
  - Projections per chunk: qT/vT from lhsT=Wqv tiles (q rows 0-63, vT rows
    64-127), kT from lhsT=Wk tiles, rhs = aT C-tiles.  Chains are kept
    contiguous on the tensor queue: interleaving two open accumulation
    groups breaks weight-load overlap (+230ns/matmul).
  - v natural [T-tile, 64|1]: all four per-chunk tiles via PE identity-
    operand transpose (128 cols each).  XBAR DMA-transposes regress: their
    ~1.2us serialized issue ops land right at the next chunk's PV start and
    stall it.  Column 64 is 1.0 (memset).
  - Scores transposed: sT[tk, tq] = lhsT kT tile [64, 128] x rhs qT chunk
    [64, 512] (contraction H=64); exp on ScalarE straight from PSUM.
    Diagonal k-tiles use their exact causal width; each diagonal tile's
    leading 128-col triangle is zeroed by one [128,128] band-mask multiply
    (DVE 2x fp16).
  - Attention per chunk: uniform per-k-tile groups — full below-diagonal
    tiles first, then the 4 diagonal tiles LAST (their exps are short, so
    the final deferred PVs are barely exposed at the kernel tail).  One
    512-col score matmul + one exp + one PV per k-tile, with PV DEFERRED
    TWO groups (queue: S0 S1 S2 P0 S3 P1 ...).  At K8 the PE outruns
    ScalarE (512-col matmul 375ns vs exp 687ns); depth 2 gives each exp
    ~1.1us of PE cover.  Score tiles are 1-PSUM-bank [128,512], bufs=3.
  - PV: outT/denom accumulate in one PSUM group per chunk: lhsT = [v | 1]
    [128, 65], rhs = expT tiles; row 64 is the softmax denominator.  No
    max-subtraction: causal logits peak ~7.2, exp <= ~1300, unnormalized
    |o| <= ~4300 and denom <= ~8800 all fit fp16.
  - NO on-device normalize: the kernel ships [o | denom] [65, T] fp16 and
    the host does out = (o/denom).T.  This removes the reciprocal/cast/
    broadcast chain (2048 PE columns + its tensor-queue stalls); the tail
    is two parallel copies (DVE+ScalarE) and stores (sync+gpsimd queues).

Timing notes (measured): minimizing total PE columns and keeping the stream
dense matters more than anything else.  fp8 DoubleRow measured only ~1.6x
per real contraction pair (cost model's 4x is wrong on this hw) and every
precision-viable fp8 construction needs residual planes that erase the gain
— fp16 everywhere is the optimum here.
"""

import sys

sys.path.insert(0, "/opt/trn_rl_repo")
sys.path.insert(0, "/root/.axon_site")

import numpy as np

import concourse.bass as bass
import concourse.mybir as mybir
import concourse.tile as tile
from concourse import bacc
from concourse import bass_utils

# If tracing is ever requested (e.g. BASS_TRACE=1), bass_utils imports
# antenv.axon_hooks, which this image lacks.  Register a ctypes-backed shim so
# that path degrades gracefully instead of raising ImportError.
try:
    from antenv import axon_hooks as _ah  # noqa: F401
except ImportError:
    try:
        import types as _types

        from trn_agent_boot.trn_boot import _ntff_profile_via_ctypes

        _mod = _types.ModuleType("antenv.axon_hooks")
        _hook = [None]
        _mod.set_axon_ntff_profile_hook = lambda h: _hook.__setitem__(0, h)
        _mod.get_axon_ntff_profile_hook = lambda: _hook[0]
        sys.modules["antenv.axon_hooks"] = _mod
        import antenv as _antenv

        _antenv.axon_hooks = _mod
        _mod.set_axon_ntff_profile_hook(
            _ntff_profile_via_ctypes("/opt/axon/libaxon_pjrt.so")
        )
    except Exception:
        pass

B, T, C, H = 8, 2048, 1024, 64
P = 128
NCT = C // P          # 8 C-tiles (contraction)
CHUNK = 512           # q-columns per chunk
NCH = T // CHUNK      # 4 chunks
NKT = T // P          # 16 k-tiles
SCALE = H ** -0.5
FP = mybir.dt.float32
F16 = mybir.dt.float16

# chunk-0 piece queues and the k-chain's piece-arrival order (measured:
# ~150 B/ns per queue early, ~1.1us completion-to-unlock lag)
Q0_SYNC = [0, 1, 4, 5]
Q0_GP = [2, 3, 6, 7]
K0_ORDER = [2, 0, 3, 1, 6, 4, 7, 5]

_cache = {}


def build_program():
    nc = bacc.Bacc("TRN2", target_bir_lowering=False, debug=False)

    aT = nc.dram_tensor("aT", [NCH, P, NCT, CHUNK], F16, kind="ExternalInput").ap()
    wqv = nc.dram_tensor("wqv", [P, NCT, 2 * H], F16, kind="ExternalInput").ap()
    wk = nc.dram_tensor("wk", [P, NCT, H], F16, kind="ExternalInput").ap()
    idh = nc.dram_tensor("idh", [P, H], F16, kind="ExternalInput").ap()
    m4 = nc.dram_tensor("m4", [P, P], F16, kind="ExternalInput").ap()
    outT = nc.dram_tensor("outT", [H + 1, T], F16, kind="ExternalOutput").ap()

    with tile.TileContext(nc) as tc:
        with (
            tc.tile_pool(name="const", bufs=1) as const_pool,
            tc.tile_pool(name="at", bufs=1) as at_pool,
            tc.tile_pool(name="qv", bufs=1) as qv_pool,
            tc.tile_pool(name="kt", bufs=1) as kt_pool,
            tc.tile_pool(name="v1", bufs=NKT) as v1_pool,
            tc.tile_pool(name="es", bufs=4) as e_pool,
            tc.tile_pool(name="out", bufs=1) as out_pool,
            tc.tile_pool(name="ps_s", bufs=3, space="PSUM") as s_psum,
            tc.tile_pool(name="ps_proj", bufs=2, space="PSUM") as proj_psum,
            tc.tile_pool(name="ps_pv", bufs=1, space="PSUM") as pv_psum,
            tc.tile_pool(name="ps_small", bufs=1, space="PSUM") as small_psum,
        ):
            # ---- warm the ACT exp table + the PE clock during the DMA window
            warm = const_pool.tile([P, 8], FP, tag="warm")
            nc.scalar.activation(
                warm[:], warm[:], mybir.ActivationFunctionType.Exp
            )
            warm2 = const_pool.tile([P, CHUNK], F16, tag="warm2")
            nc.vector.memset(warm2[:], 0.0)
            warm_ps = small_psum.tile([P, CHUNK], FP, tag="small")
            for _ in range(6):
                nc.tensor.matmul(
                    warm_ps[:], warm2[:, :P], warm2[:], start=True, stop=True,
                )

            # ---- input DMA.  Early bytes are precious: wk first on sync
            # (gates the arrival-ordered k chain), then quarter-0 single-
            # c-tile pieces split across sync and gpsimd; wqv rides sync
            # AFTER the pieces (the qv chain runs after the k chain anyway).
            # Bulk quarters ride the sync queue only: a second hwdge queue
            # on the bulk floods the 16 shared DMA rings (measured). ----
            at_sb = {}             # (j, piece-or-ctile) -> tile

            wk_sb = const_pool.tile([P, NCT, H], F16, tag="wk")
            nc.sync.dma_start(wk_sb[:], wk[:])
            for c in Q0_SYNC:
                t_ = at_pool.tile([P, 1, CHUNK], F16, tag=f"at0_{c}")
                nc.sync.dma_start(t_[:], aT[0, :, c : c + 1, :])
                at_sb[(0, c)] = t_
            for c in Q0_GP:
                t_ = at_pool.tile([P, 1, CHUNK], F16, tag=f"at0_{c}")
                nc.gpsimd.dma_start(t_[:], aT[0, :, c : c + 1, :])
                at_sb[(0, c)] = t_
            wqv_sb = const_pool.tile([P, NCT, 2 * H], F16, tag="wqv")
            nc.sync.dma_start(wqv_sb[:], wqv[:])

            at_step = {0: 1}

            def at_tile(j, c):
                step = at_step[j]
                return at_sb[(j, c // step)][:, c % step, :]

            def load_quarter(j, pieces=1):
                step = NCT // pieces
                at_step[j] = step
                for h in range(pieces):
                    t_ = at_pool.tile([P, step, CHUNK], F16, tag=f"at{j}_{h}")
                    nc.sync.dma_start(
                        t_[:], aT[j, :, h * step : (h + 1) * step, :]
                    )
                    at_sb[(j, h)] = t_

            load_quarter(1, pieces=2)
            for j in range(2, NCH):
                load_quarter(j)

            # idle-time consts on the gpsimd queue (after the ramp pieces)
            idh_sb = const_pool.tile([P, H], F16, tag="idh")
            nc.gpsimd.dma_start(idh_sb[:], idh[:])
            m4_sb = const_pool.tile([P, P], F16, tag="m4")
            nc.gpsimd.dma_start(m4_sb[:], m4[:])

            qv_sb = qv_pool.tile([P, T], F16, tag="qv")   # q rows 0-63, vT rows 64-127
            kT_sb = kt_pool.tile([H, T], F16, tag="kt")
            o65_sb = out_pool.tile([H + 1, T], F16, tag="ot")
            v1 = {}

            def proj(j):
                # NOTE: keep each PSUM accumulation chain contiguous on the
                # tensor queue — interleaving two open accumulation groups
                # costs ~230ns/matmul (weight-load overlap breaks)
                cs = slice(j * CHUNK, (j + 1) * CHUNK)
                if j == 0:
                    # DMA-paced ramp chunk: k chain first, consuming pieces
                    # in arrival order; kT copy right after so scores gate
                    # clears early; qv chain once every piece is present.
                    ps_k = proj_psum.tile([P, CHUNK], FP, tag="proj")
                    for i, c in enumerate(K0_ORDER):
                        nc.tensor.matmul(
                            ps_k[:H], wk_sb[:, c, :], at_tile(j, c),
                            start=(i == 0), stop=(i == NCT - 1),
                        )
                    nc.scalar.copy(kT_sb[:, cs], ps_k[:H])
                    ps_qv = proj_psum.tile([P, CHUNK], FP, tag="proj")
                    for c in range(NCT):
                        nc.tensor.matmul(
                            ps_qv[:], wqv_sb[:, c, :], at_tile(j, c),
                            start=(c == 0), stop=(c == NCT - 1),
                        )
                    nc.vector.tensor_copy(qv_sb[:, cs], ps_qv[:])
                else:
                    ps_qv = proj_psum.tile([P, CHUNK], FP, tag="proj")
                    for c in range(NCT):
                        nc.tensor.matmul(
                            ps_qv[:], wqv_sb[:, c, :], at_tile(j, c),
                            start=(c == 0), stop=(c == NCT - 1),
                        )
                    ps_k = proj_psum.tile([P, CHUNK], FP, tag="proj")
                    for c in range(NCT):
                        nc.tensor.matmul(
                            ps_k[:H], wk_sb[:, c, :], at_tile(j, c),
                            start=(c == 0), stop=(c == NCT - 1),
                        )
                    nc.vector.tensor_copy(qv_sb[:, cs], ps_qv[:])
                    # kT copy on ScalarE: overlaps the DVE qv copy, so
                    # scores for the next chunk are not gated on two serial
                    # DVE ops
                    nc.scalar.copy(kT_sb[:, cs], ps_k[:H])

            proj(0)
            for j in range(NCH):
                cs = slice(j * CHUNK, (j + 1) * CHUNK)

                # ---- v natural tiles ([v | 1]) via PE transpose ----
                for r in range(4):
                    kt = 4 * j + r
                    vt = v1_pool.tile([P, H + 1], F16, tag="v1")
                    nc.vector.memset(vt[:, H : H + 1], 1.0)
                    ps_t = small_psum.tile([P, H], F16, tag="small")
                    nc.tensor.transpose(
                        ps_t[:],
                        qv_sb[H:P, kt * P : (kt + 1) * P],
                        idh_sb[H:P, :],
                    )
                    nc.vector.tensor_copy(vt[:, :H], ps_t[:])
                    v1[kt] = vt

                # ---- attention: uniform per-k-tile groups, PV deferred two
                # groups (queue: S0 S1 S2 P0 S3 P1 ...).  Full k-tiles
                # first, the 4 diagonal tiles (short exps) last. ----
                ps_o = pv_psum.tile([H + 1, CHUNK], FP, tag="pv")
                order = [(kt, None) for kt in range(4 * j)]
                order += [(4 * j + r, P * r) for r in range(4)]
                n_pv = len(order)
                n_emit = 0
                pend = []

                def emit_pv():
                    nonlocal n_emit
                    args = pend.pop(0)
                    nc.tensor.matmul(
                        *args, start=(n_emit == 0), stop=(n_emit == n_pv - 1)
                    )
                    n_emit += 1

                for kt, off in order:
                    diag = off is not None
                    ncols = CHUNK - off if diag else CHUNK
                    qlo = j * CHUNK + (off or 0)
                    ps_s = s_psum.tile([P, CHUNK], FP, tag="s")
                    nc.tensor.matmul(
                        ps_s[:, :ncols],
                        kT_sb[:, kt * P : (kt + 1) * P],
                        qv_sb[:H, qlo : (j + 1) * CHUNK],
                        start=True, stop=True,
                    )
                    e_sb = e_pool.tile([P, CHUNK], F16, tag="e")
                    nc.scalar.activation(
                        e_sb[:, :ncols], ps_s[:, :ncols],
                        mybir.ActivationFunctionType.Exp,
                    )
                    if diag:
                        # zero the above-causal triangle in the leading
                        # 128-col block
                        nc.vector.tensor_mul(
                            e_sb[:, :P], e_sb[:, :P], m4_sb[:],
                        )
                    if len(pend) == 2:
                        emit_pv()
                    pend.append(
                        (
                            ps_o[:, off:] if diag else ps_o[:],
                            v1[kt][:],
                            e_sb[:, :ncols],
                        )
                    )
                while pend:
                    emit_pv()

                # ---- ship [o | denom] unnormalized; the host divides.
                # Early o65 copy releases the PV bank for the next chunk;
                # emitted BEFORE proj(j+1) so the DVE does it first. ----
                if j == NCH - 1:
                    # kernel tail: split halves across engines + parallel
                    # HWDGE queues so copies and stores drain in parallel
                    HC = CHUNK // 2
                    h0 = slice(j * CHUNK, j * CHUNK + HC)
                    h1 = slice(j * CHUNK + HC, (j + 1) * CHUNK)
                    nc.vector.tensor_copy(o65_sb[:, h0], ps_o[:, :HC])
                    nc.scalar.copy(o65_sb[:, h1], ps_o[:, HC:])
                    nc.sync.dma_start(outT[:, h0], o65_sb[:, h0])
                    nc.gpsimd.dma_start(outT[:, h1], o65_sb[:, h1])
                else:
                    nc.vector.tensor_copy(o65_sb[:, cs], ps_o[:])
                    nc.gpsimd.dma_start(outT[:, cs], o65_sb[:, cs])
                    # ---- next chunk's projections: keep the tensor queue
                    # fed while the copies run on Vector/Scalar ----
                    proj(j + 1)

    nc.compile()
    return nc


def _marshal(a, Wk, Wq, Wv):
    # [B, NCH, P, NCT, CHUNK]: quarter-major, partition-major within quarter,
    # so each partition's slice of a quarter is one contiguous 8 KiB run
    aT = np.ascontiguousarray(
        a.transpose(0, 2, 1)
        .reshape(B, NCT, P, NCH, CHUNK)
        .transpose(0, 3, 2, 1, 4)
        .astype(np.float16)
    )
    # weights pre-tiled [P, NCT, .] so each partition's DMA run is contiguous
    wqv = np.concatenate(
        [Wq * np.float32(SCALE), Wv], axis=1
    ).astype(np.float16).reshape(NCT, P, 2 * H).transpose(1, 0, 2)
    wkt = Wk.astype(np.float16).reshape(NCT, P, H).transpose(1, 0, 2)
    idh = np.zeros((P, H), np.float16)
    idh[H:P, :] = np.eye(H, dtype=np.float16)
    p = np.arange(P)[:, None]
    g = np.arange(P)[None, :]
    m4 = (g >= p).astype(np.float16)
    return (
        aT,
        np.ascontiguousarray(wqv),
        np.ascontiguousarray(wkt),
        idh,
        m4,
    )


def kernel(a, Wk, Wq, Wv):
    a = np.asarray(a, np.float32)
    Wk = np.asarray(Wk, np.float32)
    Wq = np.asarray(Wq, np.float32)
    Wv = np.asarray(Wv, np.float32)
    if "nc" not in _cache:
        _cache["nc"] = build_program()
    nc = _cache["nc"]

    aT, wqv, wk, idh, m4 = _marshal(a, Wk, Wq, Wv)
    in_maps = [
        {"aT": aT[b], "wqv": wqv, "wk": wk, "idh": idh, "m4": m4}
        for b in range(B)
    ]
    res = bass_utils.run_bass_kernel_spmd(nc, in_maps, core_ids=list(range(B)))
    outs = []
    for b in range(B):
        o = np.asarray(res.results[b]["outT"], np.float32)   # [65, T]
        outs.append((o[:H] / o[H : H + 1]).T)
    return np.stack(outs).astype(np.float32)
